# revision 9
# baseline (speedup 1.0000x reference)
"""Trainium2 Bass kernel for a vanilla transformer block (nn_BlockVanilla).

  xn  = LN(x; g1, b1)
  q,k,v = xn@Wq+bq, xn@Wk+bk, xn@Wv+bv            (H heads x E)
  h   = softmax(q k^T / sqrt(E)) v                 (per batch, per head)
  y1  = x + h@Wo + bo
  out = y1 + gelu(LN(y1; g2, b2)@W1 + bf1)@W2 + bf2

Sharding: pure data-parallel over rows.  The flattened input is [B*S, D];
core c owns rows [c*R, (c+1)*R).  Attention couples all rows of a batch, so
each core also receives its whole batch's rows ("x_batch") and computes K/V
for all of them locally (replicated-KV) — no collectives.

All matmuls run in bf16 with fp32 PSUM accumulation; LN and softmax
normalization stay in fp32.  Activations live row-major [rows(P), feat] for
LN/softmax/residual work and feature-major [feat(P), rows] as matmul
operands; 128x128 PE transposes convert between the two.  Softmax
denominators come free from a ones-column appended to V.
"""

import numpy as np

import concourse.bass as bass
import concourse.mybir as mybir
import concourse.tile as tile
from concourse import bacc
from concourse.bass_utils import run_bass_kernel_spmd
from concourse.masks import make_identity

F32 = mybir.dt.float32
BF16 = mybir.dt.bfloat16
OP = mybir.AluOpType
ACT = mybir.ActivationFunctionType

P = 128
EPS = 1e-6


def _ngroups(total, g=512):
    return [(n0, min(g, total - n0)) for n0 in range(0, total, g)]


def build_nc(R=1024, RB=2048, D=1024, H=16, E=64, FF=4096, n_cores=8,
             sim_safe_gelu=False):
    """Build the per-core Bacc graph.  R: own rows, RB: batch rows."""
    FT = D // P           # feature tiles of D
    RT = R // P           # own row tiles
    RBT = RB // P         # batch row tiles (= attention k tiles)
    FFT = FF // P         # feature tiles of FF
    HPT = P // E          # heads per feature tile
    assert H * E == D and D % P == 0 and R % P == 0 and RB % P == 0

    nc = bacc.Bacc("TRN2", target_bir_lowering=False, debug=False,
                   num_devices=n_cores)

    x_own = nc.dram_tensor("x_own", [R, D], F32, kind="ExternalInput")
    x_batch = nc.dram_tensor("x_batch", [RB, D], F32, kind="ExternalInput")
    Wq = nc.dram_tensor("Wq", [D, D], F32, kind="ExternalInput")
    Wk = nc.dram_tensor("Wk", [D, D], F32, kind="ExternalInput")
    Wv = nc.dram_tensor("Wv", [D, D], F32, kind="ExternalInput")
    Wo = nc.dram_tensor("Wo", [D, D], F32, kind="ExternalInput")
    W1 = nc.dram_tensor("W1", [D, FF], F32, kind="ExternalInput")
    W2 = nc.dram_tensor("W2", [FF, D], F32, kind="ExternalInput")
    bq = nc.dram_tensor("bq", [D], F32, kind="ExternalInput")
    bk = nc.dram_tensor("bk", [D], F32, kind="ExternalInput")
    bv = nc.dram_tensor("bv", [D], F32, kind="ExternalInput")
    bo = nc.dram_tensor("bo", [D], F32, kind="ExternalInput")
    bf1 = nc.dram_tensor("bf1", [FF], F32, kind="ExternalInput")
    bf2 = nc.dram_tensor("bf2", [D], F32, kind="ExternalInput")
    g1 = nc.dram_tensor("g1", [D], F32, kind="ExternalInput")
    b1 = nc.dram_tensor("b1", [D], F32, kind="ExternalInput")
    g2 = nc.dram_tensor("g2", [D], F32, kind="ExternalInput")
    b2 = nc.dram_tensor("b2", [D], F32, kind="ExternalInput")
    out = nc.dram_tensor("out", [R, D], F32, kind="ExternalOutput")

    inv_sqrt_e = 1.0 / float(np.sqrt(E))

    with tile.TileContext(nc) as tc:
        # --- pools with non-LIFO lifetimes: manual enter/exit ---
        def open_pool(name, bufs, space="SBUF", side="left"):
            cm = tc.tile_pool(name=name, bufs=bufs, space=space, side=side)
            return cm, cm.__enter__()

        def close_pool(cm):
            cm.__exit__(None, None, None)

        const_cm, const = open_pool("const", 1)

        ident_bf = const.tile([P, P], BF16, tag="ident_bf")
        make_identity(nc, ident_bf)
        ident_f32 = const.tile([P, P], F32, tag="ident_f32")
        make_identity(nc, ident_f32)
        eps_t = const.tile([P, 1], F32, tag="eps")
        nc.vector.memset(eps_t[:], EPS)

        # feature-major bias views [P, FT]: elem [p, f] = b[f*128+p]
        def fmaj_bias(pool, name, src, n_ft, scale=None):
            t = pool.tile([P, n_ft], F32, tag=name)
            nc.sync.dma_start(t[:], src.ap().rearrange("(f p) -> p f", p=P))
            if scale is not None:
                nc.vector.tensor_scalar_mul(t[:], t[:], scale)
            return t

        bq8_t = fmaj_bias(const, "bq8", bq, FT, scale=inv_sqrt_e)
        bk_t = fmaj_bias(const, "bk", bk, FT)
        bo_t = fmaj_bias(const, "bo", bo, FT)
        bf1_t = fmaj_bias(const, "bf1", bf1, FFT)
        bf2_t = fmaj_bias(const, "bf2", bf2, FT)

        # row-major broadcast params [P, D]
        def bcast_param(pool, name, src, n):
            row = pool.tile([1, n], F32, tag=name + "_row")
            nc.sync.dma_start(row[:], src[None, :])
            t = pool.tile([P, n], F32, tag=name)
            nc.gpsimd.partition_broadcast(t[:], row[:])
            return t

        g1_b = bcast_param(const, "g1b", g1, D)
        b1_b = bcast_param(const, "b1b", b1, D)
        bv_b = bcast_param(const, "bvb", bv, D)

        # layernorm of one row-major [P, D] fp32 tile -> bf16 transposed into
        # dstT[:, f, r*P:(r+1)*P]
        def ln_tile(xb, g_b, b_b, dstT, r, scr, stat, tps):
            nch = max(1, D // 512)
            csz = D // nch
            st6 = stat.tile([P, nch, 6], F32, tag="st6")
            for ci in range(nch):
                nc.vector.bn_stats(st6[:, ci, :], xb[:, ci * csz:(ci + 1) * csz])
            mv = stat.tile([P, 2], F32, tag="mv")
            nc.vector.bn_aggr(mv[:], st6[:])
            sd = stat.tile([P, 1], F32, tag="sd")
            nc.scalar.activation(sd[:], mv[:, 1:2], ACT.Sqrt, bias=eps_t[:])
            rstd = stat.tile([P, 1], F32, tag="rstd")
            nc.vector.reciprocal(rstd[:], sd[:])
            t1 = scr.tile([P, D], F32, tag="ln_t1")
            nc.vector.tensor_scalar(t1[:], xb[:], mv[:, 0:1], rstd[:],
                                    op0=OP.subtract, op1=OP.mult)
            nc.vector.tensor_tensor(t1[:], t1[:], g_b[:], op=OP.mult)
            xn = scr.tile([P, D], BF16, tag="ln_xn")
            nc.vector.tensor_tensor(xn[:], t1[:], b_b[:], op=OP.add)
            for f in range(FT):
                tp = tps.tile([P, P], BF16, tag="tp_bf")
                nc.tensor.transpose(tp[:], xn[:, f * P:(f + 1) * P], ident_bf[:])
                nc.vector.tensor_copy(dstT[:, f, r * P:(r + 1) * P], tp[:])

        # stream a weight chunk: DMA fp32 [P, csz], cast to bf16
        def wchunk(wpool, dram, k, c0, csz, tag):
            wf = wpool.tile([P, csz], F32, tag=tag + "_f32")
            nc.sync.dma_start(wf[:], dram[k * P:(k + 1) * P, c0:c0 + csz])
            wb = wpool.tile([P, csz], BF16, tag=tag + "_bf")
            nc.vector.tensor_copy(wb[:], wf[:])
            return wb

        # ============ Phase 1: LN1 -> xnT_b, xnT_o ============
        xnT_cm, xnT_pool = open_pool("xnT", 1)
        xnT_b = xnT_pool.tile([P, FT, RB], BF16, tag="xnT_b")
        xnT_o = xnT_pool.tile([P, FT, R], BF16, tag="xnT_o")
        with tc.tile_pool(name="ln_x", bufs=2) as xpool, \
             tc.tile_pool(name="ln_scr", bufs=2) as scr, \
             tc.tile_pool(name="ln_stat", bufs=4) as stat, \
             tc.tile_pool(name="tps1", bufs=4, space="PSUM") as tps:
            for r in range(RBT):
                xb = xpool.tile([P, D], F32, tag="ln_x")
                nc.sync.dma_start(xb[:], x_batch[r * P:(r + 1) * P, :])
                ln_tile(xb, g1_b, b1_b, xnT_b, r, scr, stat, tps)
            for r in range(RT):
                xb = xpool.tile([P, D], F32, tag="ln_x")
                nc.sync.dma_start(xb[:], x_own[r * P:(r + 1) * P, :])
                ln_tile(xb, g1_b, b1_b, xnT_o, r, scr, stat, tps)
        # ============ Phase 2: QKV ============
        att_cm, att_pool = open_pool("att", 1, side="right")
        kT = att_pool.tile([P, FT, RB], BF16, tag="kT")
        v_aug = att_pool.tile([P, RBT, H * (E + 1)], BF16, tag="v_aug")
        qT = att_pool.tile([P, FT, R], BF16, tag="qT")

        with tc.tile_pool(name="w_qkv", bufs=4) as wpool, \
             tc.tile_pool(name="mm2", bufs=8, space="PSUM") as mm:

            # kT (feature-major):  kT[f, :] = sum_k Wk[k,f].T @ xnT_b[k, :]
            for f in range(FT):
                groups = _ngroups(RB)
                pss = [mm.tile([P, nsz], F32, name="mm2", tag="mm2") for (_, nsz) in groups]
                for k in range(FT):
                    wb = wchunk(wpool, Wk, k, f * P, P, "wk")
                    for ni, (n0, nsz) in enumerate(groups):
                        nc.tensor.matmul(pss[ni][:], wb[:],
                                         xnT_b[:, k, n0:n0 + nsz],
                                         start=(k == 0), stop=(k == FT - 1))
                for ni, (n0, nsz) in enumerate(groups):
                    nc.vector.tensor_scalar(kT[:, f, n0:n0 + nsz], pss[ni][:],
                                            bk_t[:, f:f + 1], None, op0=OP.add)

            # qT with 1/sqrt(E) folded in:  (psum + bq) * inv_sqrt_e
            for f in range(FT):
                groups = _ngroups(R)
                pss = [mm.tile([P, nsz], F32, name="mm2", tag="mm2") for (_, nsz) in groups]
                for k in range(FT):
                    wb = wchunk(wpool, Wq, k, f * P, P, "wq")
                    for ni, (n0, nsz) in enumerate(groups):
                        nc.tensor.matmul(pss[ni][:], wb[:],
                                         xnT_o[:, k, n0:n0 + nsz],
                                         start=(k == 0), stop=(k == FT - 1))
                for ni, (n0, nsz) in enumerate(groups):
                    nc.vector.tensor_scalar(qT[:, f, n0:n0 + nsz], pss[ni][:],
                                            inv_sqrt_e, bq8_t[:, f:f + 1],
                                            op0=OP.mult, op1=OP.add)

            # v (row-major, per-head augmented with a ones column)
            wv_cm, wv_pool = open_pool("wv", 1)
            Wv_bf = wv_pool.tile([P, FT, D], BF16, tag="Wv_bf")
            for k in range(FT):
                wf = wpool.tile([P, D], F32, tag="wv_f32")
                nc.sync.dma_start(wf[:], Wv[k * P:(k + 1) * P, :])
                nc.vector.tensor_copy(Wv_bf[:, k, :], wf[:])
            for t in range(RBT):
                groups = _ngroups(D)
                pss = [mm.tile([P, nsz], F32, name="mm2", tag="mm2") for (_, nsz) in groups]
                for k in range(FT):
                    for ni, (n0, nsz) in enumerate(groups):
                        nc.tensor.matmul(pss[ni][:],
                                         xnT_b[:, k, t * P:(t + 1) * P],
                                         Wv_bf[:, k, n0:n0 + nsz],
                                         start=(k == 0), stop=(k == FT - 1))
                va = v_aug[:, t, :].rearrange("p (h e) -> p h e", e=E + 1)
                for ni, (n0, nsz) in enumerate(groups):
                    hs = n0 // E
                    nh = nsz // E
                    nc.vector.tensor_tensor(
                        va[:, hs:hs + nh, 0:E],
                        pss[ni].rearrange("p (h e) -> p h e", e=E),
                        bv_b[:, n0:n0 + nsz].rearrange("p (h e) -> p h e", e=E),
                        op=OP.add)
                nc.vector.memset(va[:, :, E:E + 1], 1.0)
            close_pool(wv_cm)
        close_pool(xnT_cm)

        # ============ Phase 3: attention ============
        hT_cm, hT_pool = open_pool("hT", 1)
        hT = hT_pool.tile([P, FT, R], BF16, tag="hT")

        qgroups = _ngroups(R)
        with tc.tile_pool(name="spsum", bufs=4, space="PSUM") as spool, \
             tc.tile_pool(name="opsum", bufs=2 * len(qgroups), space="PSUM") as opool, \
             tc.tile_pool(name="expool", bufs=RBT * len(qgroups) + 4) as expool, \
             tc.tile_pool(name="attn_n", bufs=3) as npool:
            for h in range(H):
                f_h = h // HPT
                p_h = (h % HPT) * E
                o_pss = [opool.tile([E + 1, qsz], F32, name="o", tag="o")
                         for (_, qsz) in qgroups]
                exs = {}
                for t in range(RBT):
                    for qi, (q0, qsz) in enumerate(qgroups):
                        s_ps = spool.tile([P, qsz], F32, tag="s")
                        nc.tensor.matmul(
                            s_ps[:],
                            kT[p_h:p_h + E, f_h, t * P:(t + 1) * P],
                            qT[p_h:p_h + E, f_h, q0:q0 + qsz],
                            start=True, stop=True)
                        ex = expool.tile([P, qsz], BF16, tag="ex")
                        nc.scalar.activation(ex[:], s_ps[:], ACT.Exp)
                        exs[(t, qi)] = ex
                va = v_aug[:, :, :].rearrange("p t (h e) -> p t h e", e=E + 1)
                for t in range(RBT):
                    for qi, (q0, qsz) in enumerate(qgroups):
                        nc.tensor.matmul(o_pss[qi][:], va[:, t, h, :],
                                         exs[(t, qi)][:],
                                         start=(t == 0), stop=(t == RBT - 1))
                for qi, (q0, qsz) in enumerate(qgroups):
                    rec = npool.tile([1, qsz], F32, tag="rec")
                    nc.vector.reciprocal(rec[:], o_pss[qi][E:E + 1, :])
                    bcst = npool.tile([E, qsz], F32, tag="bc")
                    nc.gpsimd.partition_broadcast(bcst[:], rec[:])
                    nc.vector.tensor_tensor(hT[p_h:p_h + E, f_h, q0:q0 + qsz],
                                            o_pss[qi][0:E, :], bcst[:],
                                            op=OP.mult)
        close_pool(att_cm)

        # ============ Phase 4: Wo projection + residual ============
        y1_cm, y1_pool = open_pool("y1", 1, side="right")
        y1 = y1_pool.tile([P, RT, D], F32, tag="y1")
        with tc.tile_pool(name="w_o", bufs=4) as wpool4, \
             tc.tile_pool(name="mm4", bufs=4, space="PSUM") as mm4, \
             tc.tile_pool(name="tp4", bufs=4, space="PSUM") as tp4, \
             tc.tile_pool(name="ev4", bufs=3) as ev4, \
             tc.tile_pool(name="xres", bufs=4) as xres:
            for f in range(FT):
                groups = _ngroups(R)
                pss = [mm4.tile([P, nsz], F32, name="mm4", tag="mm4") for (_, nsz) in groups]
                for k in range(FT):
                    wb = wchunk(wpool4, Wo, k, f * P, P, "wo")
                    for ni, (n0, nsz) in enumerate(groups):
                        nc.tensor.matmul(pss[ni][:], wb[:], hT[:, k, n0:n0 + nsz],
                                         start=(k == 0), stop=(k == FT - 1))
                for ni, (n0, nsz) in enumerate(groups):
                    pe = ev4.tile([P, nsz], F32, tag="pe")
                    nc.vector.tensor_scalar(pe[:], pss[ni][:],
                                            bo_t[:, f:f + 1], None, op0=OP.add)
                    for j in range(nsz // P):
                        tp = tp4.tile([P, P], F32, tag="tp32")
                        nc.tensor.transpose(tp[:], pe[:, j * P:(j + 1) * P],
                                            ident_f32[:])
                        r = (n0 + j * P) // P
                        xo = xres.tile([P, P], F32, tag="xo")
                        nc.sync.dma_start(
                            xo[:], x_own[r * P:(r + 1) * P, f * P:(f + 1) * P])
                        nc.vector.tensor_tensor(y1[:, r, f * P:(f + 1) * P],
                                                tp[:], xo[:], op=OP.add)
        close_pool(hT_cm)

        # ============ Phase 5: LN2 -> xn2T ============
        xn2_cm, xn2_pool = open_pool("xn2T", 1)
        xn2T = xn2_pool.tile([P, FT, R], BF16, tag="xn2T")
        g2_b = bcast_param(const, "g2b", g2, D)
        b2_b = bcast_param(const, "b2b", b2, D)
        with tc.tile_pool(name="ln_scr2", bufs=2) as scr2, \
             tc.tile_pool(name="ln_stat2", bufs=4) as stat2, \
             tc.tile_pool(name="tps5", bufs=4, space="PSUM") as tps5:
            for r in range(RT):
                ln_tile(y1[:, r, :], g2_b, b2_b, xn2T, r, scr2, stat2, tps5)

        # ============ Phase 6: FFN up + gelu ============
        ff1_cm, ff1_pool = open_pool("ff1T", 1)
        ff1T = ff1_pool.tile([P, FFT, R], BF16, tag="ff1T")
        with tc.tile_pool(name="w_1", bufs=4) as wpool6, \
             tc.tile_pool(name="mm6", bufs=8, space="PSUM") as mm6:
            for fb in range(0, FFT, 2):
                groups = _ngroups(R)
                pss = {}
                for mi in range(2):
                    for ni, (n0, nsz) in enumerate(groups):
                        pss[(mi, ni)] = mm6.tile([P, nsz], F32, name="mm6", tag="mm6")
                for k in range(FT):
                    wb = wchunk(wpool6, W1, k, fb * P, 2 * P, "w1")
                    for mi in range(2):
                        for ni, (n0, nsz) in enumerate(groups):
                            nc.tensor.matmul(pss[(mi, ni)][:],
                                             wb[:, mi * P:(mi + 1) * P],
                                             xn2T[:, k, n0:n0 + nsz],
                                             start=(k == 0), stop=(k == FT - 1))
                for mi in range(2):
                    f = fb + mi
                    for ni, (n0, nsz) in enumerate(groups):
                        if not sim_safe_gelu:
                            nc.scalar.activation(ff1T[:, f, n0:n0 + nsz],
                                                 pss[(mi, ni)][:], ACT.Gelu,
                                                 bias=bf1_t[:, f:f + 1])
                        else:
                            _gelu_tanh(nc, tc, ff1T[:, f, n0:n0 + nsz],
                                       pss[(mi, ni)][:], bf1_t[:, f:f + 1],
                                       P, nsz)

        # ============ Phase 7: FFN down + residual -> out ============
        with tc.tile_pool(name="w_2", bufs=4) as wpool7, \
             tc.tile_pool(name="mm7", bufs=4, space="PSUM") as mm7, \
             tc.tile_pool(name="tp7", bufs=4, space="PSUM") as tp7, \
             tc.tile_pool(name="ev7", bufs=3) as ev7, \
             tc.tile_pool(name="ob7", bufs=4) as ob7:
            for fb in range(0, FT, 2):
                groups = _ngroups(R)
                pss = {}
                for mi in range(2):
                    for ni, (n0, nsz) in enumerate(groups):
                        pss[(mi, ni)] = mm7.tile([P, nsz], F32, name="mm7", tag="mm7")
                for k in range(FFT):
                    wb = wchunk(wpool7, W2, k, fb * P, 2 * P, "w2")
                    for mi in range(2):
                        for ni, (n0, nsz) in enumerate(groups):
                            nc.tensor.matmul(pss[(mi, ni)][:],
                                             wb[:, mi * P:(mi + 1) * P],
                                             ff1T[:, k, n0:n0 + nsz],
                                             start=(k == 0), stop=(k == FFT - 1))
                for mi in range(2):
                    f = fb + mi
                    for ni, (n0, nsz) in enumerate(groups):
                        pe = ev7.tile([P, nsz], F32, tag="pe7")
                        nc.vector.tensor_scalar(pe[:], pss[(mi, ni)][:],
                                                bf2_t[:, f:f + 1], None,
                                                op0=OP.add)
                        for j in range(nsz // P):
                            tp = tp7.tile([P, P], F32, tag="tp7")
                            nc.tensor.transpose(tp[:], pe[:, j * P:(j + 1) * P],
                                                ident_f32[:])
                            r = (n0 + j * P) // P
                            ob = ob7.tile([P, P], F32, tag="ob")
                            nc.vector.tensor_tensor(
                                ob[:], tp[:], y1[:, r, f * P:(f + 1) * P],
                                op=OP.add)
                            nc.sync.dma_start(
                                out[r * P:(r + 1) * P, f * P:(f + 1) * P], ob[:])
        close_pool(ff1_cm)
        close_pool(xn2_cm)
        close_pool(y1_cm)
        close_pool(const_cm)

    nc.compile()
    return nc


def _gelu_tanh(nc, tc, out_ap, ps, bias_col, p, nsz):
    """CoreSim-safe tanh gelu: 0.5*x*(1+tanh(0.79788456*(x+0.044715*x^3)))."""
    with tc.tile_pool(name="gelu_scr", bufs=2) as gs:
        x = gs.tile([p, nsz], F32, tag="g_x")
        nc.vector.tensor_scalar(x[:], ps[:], bias_col, None, op0=OP.add)
        x3 = gs.tile([p, nsz], F32, tag="g_x3")
        nc.vector.tensor_tensor(x3[:], x[:], x[:], op=OP.mult)
        nc.vector.tensor_tensor(x3[:], x3[:], x[:], op=OP.mult)
        nc.vector.tensor_scalar(x3[:], x3[:], 0.044715, None, op0=OP.mult)
        nc.vector.tensor_tensor(x3[:], x3[:], x[:], op=OP.add)
        th = gs.tile([p, nsz], F32, tag="g_th")
        nc.scalar.activation(th[:], x3[:], ACT.Tanh, scale=0.7978845608028654)
        nc.vector.tensor_scalar(th[:], th[:], 1.0, 0.5, op0=OP.add, op1=OP.mult)
        nc.vector.tensor_tensor(out_ap, x[:], th[:], op=OP.mult)


# ---------------- host-side driver ----------------

_COMPILED = {}

_B, _S, _D, _H, _E, _FF = 4, 2048, 1024, 16, 64, 4096
_NCORES = 8
_R = (_B * _S) // _NCORES          # 1024 own rows per core
_CPB = _NCORES // _B               # cores per batch


def _get_nc():
    key = "full"
    if key not in _COMPILED:
        _COMPILED[key] = build_nc(R=_R, RB=_S, D=_D, H=_H, E=_E, FF=_FF,
                                  n_cores=_NCORES)
    return _COMPILED[key]


_WNAMES = ["Wq", "Wk", "Wv", "Wo", "W1", "W2", "bq", "bk", "bv", "bo",
           "bf1", "bf2", "g1", "b1", "g2", "b2"]


def kernel(**inputs):
    nc = _get_nc()
    x = np.ascontiguousarray(np.asarray(inputs["x"], dtype=np.float32))
    xf = x.reshape(_NCORES, _R, _D)
    xb = x.reshape(_B, _S, _D)
    shared = {n: np.ascontiguousarray(np.asarray(inputs[n], dtype=np.float32))
              for n in _WNAMES}
    in_maps = []
    for c in range(_NCORES):
        m = dict(shared)
        m["x_own"] = xf[c]
        m["x_batch"] = xb[c // _CPB]
        in_maps.append(m)
    res = run_bass_kernel_spmd(nc, in_maps, core_ids=list(range(_NCORES)))
    out = np.concatenate([res.results[c]["out"] for c in range(_NCORES)], axis=0)
    return out.reshape(_B, _S, _D).astype(np.float32)


# revision 17
# speedup vs baseline: 1.3157x; 1.3157x over previous
"""Trainium2 Bass kernel for a vanilla transformer block (nn_BlockVanilla).

  xn  = LN(x; g1, b1)
  q,k,v = xn@Wq+bq, xn@Wk+bk, xn@Wv+bv            (H heads x E)
  h   = softmax(q k^T / sqrt(E)) v                 (per batch, per head)
  y1  = x + h@Wo + bo
  out = y1 + gelu(LN(y1; g2, b2)@W1 + bf1)@W2 + bf2

Sharding: pure data-parallel over rows.  The flattened input is [B*S, D];
core c owns rows [c*R, (c+1)*R).  Attention couples all rows of a batch, so
each core also receives its whole batch's rows ("x_batch") and computes K/V
for all of them locally (replicated-KV) — no collectives.

All matmuls run in bf16 with fp32 PSUM accumulation; LN and softmax
normalization stay in fp32.  Activations live row-major [rows(P), feat] for
LN/softmax/residual work and feature-major [feat(P), rows] as matmul
operands; 128x128 PE transposes convert between the two.  Softmax
denominators come free from a ones-column appended to V.

Scheduling notes (v2): V-projection matmuls are interleaved into the LN1
loop so the TensorEngine never idles long enough for the HAM clock gate to
re-throttle; transpose evictions are batched 4-wide; attention exp() is
batched 2 k-tiles per ACTIVATE; softmax normalization uses one
reciprocal_approx_fast per (head, q-group); weight fp32->bf16 casts go to
whichever of ScalarE/VectorE is idle in that phase.
"""

import numpy as np

import concourse.bass as bass
import concourse.mybir as mybir
import concourse.tile as tile
from concourse import bacc
from concourse.bass_utils import run_bass_kernel_spmd
from concourse.masks import make_identity

F32 = mybir.dt.float32
BF16 = mybir.dt.bfloat16
OP = mybir.AluOpType
ACT = mybir.ActivationFunctionType

P = 128
EPS = 1e-6


def _ngroups(total, g=512):
    return [(n0, min(g, total - n0)) for n0 in range(0, total, g)]


def build_nc(R=1024, RB=2048, D=1024, H=16, E=64, FF=4096, n_cores=8,
             sim_safe_gelu=False):
    """Build the per-core Bacc graph.  R: own rows, RB: batch rows."""
    FT = D // P           # feature tiles of D
    RT = R // P           # own row tiles
    RBT = RB // P         # batch row tiles (= attention k tiles)
    FFT = FF // P         # feature tiles of FF
    HPT = P // E          # heads per feature tile
    assert H * E == D and D % P == 0 and R % P == 0 and RB % P == 0

    nc = bacc.Bacc("TRN2", target_bir_lowering=False, debug=False,
                   num_devices=n_cores)

    x_own = nc.dram_tensor("x_own", [R, D], F32, kind="ExternalInput")
    x_batch = nc.dram_tensor("x_batch", [RB, D], F32, kind="ExternalInput")
    Wq = nc.dram_tensor("Wq", [D, D], F32, kind="ExternalInput")
    Wk = nc.dram_tensor("Wk", [D, D], F32, kind="ExternalInput")
    Wv = nc.dram_tensor("Wv", [D, D], F32, kind="ExternalInput")
    Wo = nc.dram_tensor("Wo", [D, D], F32, kind="ExternalInput")
    W1 = nc.dram_tensor("W1", [D, FF], F32, kind="ExternalInput")
    W2 = nc.dram_tensor("W2", [FF, D], F32, kind="ExternalInput")
    bq = nc.dram_tensor("bq", [D], F32, kind="ExternalInput")
    bk = nc.dram_tensor("bk", [D], F32, kind="ExternalInput")
    bv = nc.dram_tensor("bv", [D], F32, kind="ExternalInput")
    bo = nc.dram_tensor("bo", [D], F32, kind="ExternalInput")
    bf1 = nc.dram_tensor("bf1", [FF], F32, kind="ExternalInput")
    bf2 = nc.dram_tensor("bf2", [D], F32, kind="ExternalInput")
    g1 = nc.dram_tensor("g1", [D], F32, kind="ExternalInput")
    b1 = nc.dram_tensor("b1", [D], F32, kind="ExternalInput")
    g2 = nc.dram_tensor("g2", [D], F32, kind="ExternalInput")
    b2 = nc.dram_tensor("b2", [D], F32, kind="ExternalInput")
    out = nc.dram_tensor("out", [R, D], F32, kind="ExternalOutput")

    inv_sqrt_e = 1.0 / float(np.sqrt(E))

    with tile.TileContext(nc) as tc:
        # --- pools with non-LIFO lifetimes: manual enter/exit (per side) ---
        def open_pool(name, bufs, space="SBUF", side="left"):
            cm = tc.tile_pool(name=name, bufs=bufs, space=space, side=side)
            return cm, cm.__enter__()

        def close_pool(cm):
            cm.__exit__(None, None, None)

        const_cm, const = open_pool("const", 1)

        ident_bf = const.tile([P, P], BF16, tag="ident_bf")
        make_identity(nc, ident_bf)
        ident_f32 = const.tile([P, P], F32, tag="ident_f32")
        make_identity(nc, ident_f32)
        eps_t = const.tile([P, 1], F32, tag="eps")
        nc.vector.memset(eps_t[:], EPS)
        ones_e = const.tile([P, E], BF16, tag="ones_e")
        nc.vector.memset(ones_e[:], 1.0)

        # feature-major bias views [P, FT]: elem [p, f] = b[f*128+p]
        def fmaj_bias(pool, name, src, n_ft, scale=None):
            t = pool.tile([P, n_ft], F32, tag=name, name=name)
            nc.sync.dma_start(t[:], src.ap().rearrange("(f p) -> p f", p=P))
            if scale is not None:
                nc.vector.tensor_scalar_mul(t[:], t[:], scale)
            return t

        bq8_t = fmaj_bias(const, "bq8", bq, FT, scale=inv_sqrt_e)
        bk_t = fmaj_bias(const, "bk", bk, FT)
        bo_t = fmaj_bias(const, "bo", bo, FT)
        bf1_t = fmaj_bias(const, "bf1", bf1, FFT)
        bf2_t = fmaj_bias(const, "bf2", bf2, FT)

        # row-major broadcast params [P, D] (stored bf16; staging rows shared)
        def bcast_param(pool, name, src, n):
            row = pool.tile([1, n], F32, tag="row_f32", name=name)
            nc.sync.dma_start(row[:], src[None, :])
            rb = pool.tile([1, n], BF16, tag="row_bf", name=name)
            nc.vector.tensor_copy(rb[:], row[:])
            t = pool.tile([P, n], BF16, tag=name, name=name)
            nc.gpsimd.partition_broadcast(t[:], rb[:])
            return t

        g1_b = bcast_param(const, "g1b", g1, D)
        b1_b = bcast_param(const, "b1b", b1, D)
        g2_b = bcast_param(const, "g2b", g2, D)
        b2_b = bcast_param(const, "b2b", b2, D)
        bv_b = bcast_param(const, "bvb", bv, D)

        # layernorm of one row-major [P, D] fp32 tile -> bf16, transposed into
        # dstT[:, f, r*P:(r+1)*P].  Transpose evictions batched 4 f's wide.
        def ln_tile(xb, g_b, b_b, dstT, r, scr, stat, tps):
            nch = max(1, D // 512)
            csz = D // nch
            st6 = stat.tile([P, nch, 6], F32, tag="st6", name="st6")
            for ci in range(nch):
                nc.vector.bn_stats(st6[:, ci, :], xb[:, ci * csz:(ci + 1) * csz])
            mv = stat.tile([P, 2], F32, tag="mv", name="mv")
            nc.vector.bn_aggr(mv[:], st6[:])
            sd = stat.tile([P, 1], F32, tag="sd", name="sd")
            nc.scalar.activation(sd[:], mv[:, 1:2], ACT.Sqrt, bias=eps_t[:])
            rstd = stat.tile([P, 1], F32, tag="rstd", name="rstd")
            nc.vector.reciprocal(rstd[:], sd[:])
            t1 = scr.tile([P, D], F32, tag="ln_t1", name="ln_t1")
            nc.vector.tensor_scalar(t1[:], xb[:], mv[:, 0:1], rstd[:],
                                    op0=OP.subtract, op1=OP.mult)
            nc.vector.tensor_tensor(t1[:], t1[:], g_b[:], op=OP.mult)
            xn = scr.tile([P, D], BF16, tag="ln_xn", name="ln_xn")
            nc.vector.tensor_tensor(xn[:], t1[:], b_b[:], op=OP.add)
            for fb in range(0, FT, 4):
                nf = min(4, FT - fb)
                tp = tps.tile([P, nf * P], BF16, tag="tp_bf", name="tp_bf")
                for j in range(nf):
                    nc.tensor.transpose(tp[:, j * P:(j + 1) * P],
                                        xn[:, (fb + j) * P:(fb + j + 1) * P],
                                        ident_bf[:])
                nc.vector.tensor_copy(
                    dstT[:, fb:fb + nf, r * P:(r + 1) * P],
                    tp.rearrange("p (f c) -> p f c", c=P))

        # stream a weight chunk: DMA fp32 [P, csz], cast to bf16 on `eng`
        def wchunk(wpool, dram, k, c0, csz, tag, eng="scalar"):
            wf = wpool.tile([P, csz], F32, tag=tag + "_f32", name=tag)
            nc.sync.dma_start(wf[:], dram[k * P:(k + 1) * P, c0:c0 + csz])
            wb = wpool.tile([P, csz], BF16, tag=tag + "_bf", name=tag)
            if eng == "scalar":
                nc.scalar.activation(wb[:], wf[:], ACT.Copy)
            else:
                nc.vector.tensor_copy(wb[:], wf[:])
            return wb

        # ============ Phase 1+2: LN1, V (interleaved), K, Q ============
        xnT_cm, xnT_pool = open_pool("xnT", 1)
        xnT_b = xnT_pool.tile([P, FT, RB], BF16, tag="xnT_b")
        xnT_o = xnT_pool.tile([P, FT, R], BF16, tag="xnT_o")
        att_cm, att_pool = open_pool("att", 1, side="right")
        kT = att_pool.tile([P, FT, RB], BF16, tag="kT")
        v_aug = att_pool.tile([P, RBT, H * (E + 1)], BF16, tag="v_aug")
        qT = att_pool.tile([P, FT, R], BF16, tag="qT")
        wv_cm, wv_pool = open_pool("wv", 1)
        Wv_bf = wv_pool.tile([P, FT, D], BF16, tag="Wv_bf")

        with tc.tile_pool(name="ln_x", bufs=2) as xpool, \
             tc.tile_pool(name="ln_scr", bufs=2) as scr, \
             tc.tile_pool(name="ln_stat", bufs=4) as stat, \
             tc.tile_pool(name="w_qkv", bufs=3) as wpool, \
             tc.tile_pool(name="tps1", bufs=3, space="PSUM") as tps, \
             tc.tile_pool(name="mm2", bufs=4, space="PSUM") as mm:

            # Wv upfront (V matmuls run inside the LN1 loop)
            for k in range(FT):
                for c0, csz in _ngroups(D):
                    wf = wpool.tile([P, csz], F32, tag="wv_f32", name="wv",
                                    bufs=2)
                    nc.sync.dma_start(wf[:], Wv[k * P:(k + 1) * P, c0:c0 + csz])
                    nc.scalar.activation(Wv_bf[:, k, c0:c0 + csz], wf[:],
                                         ACT.Copy)

            vgroups = _ngroups(D)
            for t in range(RBT):
                xb = xpool.tile([P, D], F32, tag="ln_x", name="ln_x")
                nc.sync.dma_start(xb[:], x_batch[t * P:(t + 1) * P, :])
                ln_tile(xb, g1_b, b1_b, xnT_b, t, scr, stat, tps)
                # V for this row tile (row-major, per-head ones column)
                pss = [mm.tile([P, nsz], F32, name="mm2", tag="mm2")
                       for (_, nsz) in vgroups]
                for k in range(FT):
                    for ni, (n0, nsz) in enumerate(vgroups):
                        nc.tensor.matmul(pss[ni][:],
                                         xnT_b[:, k, t * P:(t + 1) * P],
                                         Wv_bf[:, k, n0:n0 + nsz],
                                         start=(k == 0), stop=(k == FT - 1))
                va = v_aug[:, t, :].rearrange("p (h e) -> p h e", e=E + 1)
                for ni, (n0, nsz) in enumerate(vgroups):
                    hs = n0 // E
                    nh = nsz // E
                    nc.vector.tensor_tensor(
                        va[:, hs:hs + nh, 0:E],
                        pss[ni].rearrange("p (h e) -> p h e", e=E),
                        bv_b[:, n0:n0 + nsz].rearrange("p (h e) -> p h e", e=E),
                        op=OP.add)
                nc.vector.memset(va[:, :, E:E + 1], 1.0)

            # LN of own rows (DVE) overlaps the kT matmuls below (PE)
            for r in range(RT):
                xb = xpool.tile([P, D], F32, tag="ln_x", name="ln_x")
                nc.sync.dma_start(xb[:], x_own[r * P:(r + 1) * P, :])
                ln_tile(xb, g1_b, b1_b, xnT_o, r, scr, stat, tps)

            # kT (feature-major)
            kgroups = _ngroups(RB)
            for f in range(FT):
                pss = [mm.tile([P, nsz], F32, name="mm2", tag="mm2")
                       for (_, nsz) in kgroups]
                for k in range(FT):
                    wb = wchunk(wpool, Wk, k, f * P, P, "wk", eng="scalar")
                    for ni, (n0, nsz) in enumerate(kgroups):
                        nc.tensor.matmul(pss[ni][:], wb[:],
                                         xnT_b[:, k, n0:n0 + nsz],
                                         start=(k == 0), stop=(k == FT - 1))
                for ni, (n0, nsz) in enumerate(kgroups):
                    nc.vector.tensor_scalar(kT[:, f, n0:n0 + nsz], pss[ni][:],
                                            bk_t[:, f:f + 1], None, op0=OP.add)

            # qT with 1/sqrt(E) folded in
            qgroups = _ngroups(R)
            for f in range(FT):
                pss = [mm.tile([P, nsz], F32, name="mm2", tag="mm2")
                       for (_, nsz) in qgroups]
                for k in range(FT):
                    wb = wchunk(wpool, Wq, k, f * P, P, "wq", eng="scalar")
                    for ni, (n0, nsz) in enumerate(qgroups):
                        nc.tensor.matmul(pss[ni][:], wb[:],
                                         xnT_o[:, k, n0:n0 + nsz],
                                         start=(k == 0), stop=(k == FT - 1))
                for ni, (n0, nsz) in enumerate(qgroups):
                    nc.vector.tensor_scalar(qT[:, f, n0:n0 + nsz], pss[ni][:],
                                            inv_sqrt_e, bq8_t[:, f:f + 1],
                                            op0=OP.mult, op1=OP.add)
        close_pool(wv_cm)
        close_pool(xnT_cm)

        # ============ Phase 3: attention ============
        hT_cm, hT_pool = open_pool("hT", 1)
        hT = hT_pool.tile([P, FT, R], BF16, tag="hT")

        qgroups = _ngroups(R)
        NQ = len(qgroups)
        with tc.tile_pool(name="spsum", bufs=3, space="PSUM") as spool, \
             tc.tile_pool(name="opsum", bufs=NQ, space="PSUM") as opool, \
             tc.tile_pool(name="expool", bufs=6) as expool, \
             tc.tile_pool(name="attn_n", bufs=3) as npool:
            va4 = v_aug.rearrange("p t (h e) -> p t h e", e=E + 1)
            for h in range(H):
                f_h = h // HPT
                p_h = (h % HPT) * E
                o_pss = [opool.tile([E + 1, qsz], F32, name="o", tag="o")
                         for (_, qsz) in qgroups]
                for tb in range(0, RBT, 2):
                    s2s = [spool.tile([P, 2, qsz], F32, name="s2", tag="s2")
                           for (_, qsz) in qgroups]
                    for dt in range(2):
                        t = tb + dt
                        for qi, (q0, qsz) in enumerate(qgroups):
                            nc.tensor.matmul(
                                s2s[qi][:, dt, :],
                                kT[p_h:p_h + E, f_h, t * P:(t + 1) * P],
                                qT[p_h:p_h + E, f_h, q0:q0 + qsz],
                                start=True, stop=True)
                    ex2s = []
                    for qi, (q0, qsz) in enumerate(qgroups):
                        ex2 = expool.tile([P, 2, qsz], BF16, name="ex", tag="ex")
                        nc.scalar.activation(ex2[:], s2s[qi][:], ACT.Exp)
                        ex2s.append(ex2)
                    for dt in range(2):
                        t = tb + dt
                        for qi, (q0, qsz) in enumerate(qgroups):
                            nc.tensor.matmul(o_pss[qi][:], va4[:, t, h, :],
                                             ex2s[qi][:, dt, :],
                                             start=(t == 0), stop=(t == RBT - 1))
                for qi, (q0, qsz) in enumerate(qgroups):
                    rec = npool.tile([1, qsz], F32, name="rec", tag="rec")
                    nc.vector.reciprocal(rec[:], o_pss[qi][E:E + 1, :])
                    bcst = npool.tile([E, qsz], F32, name="bc", tag="bc")
                    nc.gpsimd.partition_broadcast(bcst[:], rec[:])
                    nc.vector.tensor_tensor(hT[p_h:p_h + E, f_h, q0:q0 + qsz],
                                            o_pss[qi][0:E, :], bcst[:],
                                            op=OP.mult)
        close_pool(att_cm)

        # ============ Phase 4+5: Wo projection + residual, LN2 (pipelined) ==
        y1_cm, y1_pool = open_pool("y1", 1, side="right")
        y1 = y1_pool.tile([P, RT, D], F32, tag="y1")
        xn2_cm, xn2_pool = open_pool("xn2T", 1, side="right")
        xn2T = xn2_pool.tile([P, FT, R], BF16, tag="xn2T")

        with tc.tile_pool(name="w_o", bufs=4) as wpool4, \
             tc.tile_pool(name="ln_scr2", bufs=3) as scr2, \
             tc.tile_pool(name="ln_stat2", bufs=4) as stat2, \
             tc.tile_pool(name="ev4", bufs=3) as ev4, \
             tc.tile_pool(name="xres", bufs=3) as xres, \
             tc.tile_pool(name="mm4", bufs=4, space="PSUM") as mm4, \
             tc.tile_pool(name="tp4", bufs=2, space="PSUM") as tp4:
            for ni, (n0, nsz) in enumerate(_ngroups(R)):
                nj = nsz // P
                for f in range(FT):
                    ps = mm4.tile([P, nsz], F32, name="mm4", tag="mm4")
                    for k in range(FT):
                        wb = wchunk(wpool4, Wo, k, f * P, P, "wo", eng="scalar")
                        nc.tensor.matmul(ps[:], wb[:], hT[:, k, n0:n0 + nsz],
                                         start=(k == 0), stop=(k == FT - 1))
                    pe = ev4.tile([P, nsz], F32, name="pe", tag="pe")
                    nc.vector.tensor_scalar(pe[:], ps[:], bo_t[:, f:f + 1],
                                            None, op0=OP.add)
                    tp = tp4.tile([P, nsz], F32, name="tp4", tag="tp4")
                    for j in range(nj):
                        nc.tensor.transpose(tp[:, j * P:(j + 1) * P],
                                            pe[:, j * P:(j + 1) * P],
                                            ident_f32[:])
                    xo = xres.tile([P, nj, P], F32, name="xo", tag="xo")
                    nc.sync.dma_start(
                        xo[:], x_own[n0:n0 + nsz, f * P:(f + 1) * P]
                        .rearrange("(j p) c -> p j c", p=P))
                    nc.vector.tensor_tensor(
                        y1[:, n0 // P:n0 // P + nj, f * P:(f + 1) * P],
                        tp.rearrange("p (j c) -> p j c", c=P), xo[:], op=OP.add)
                # LN2 for the rows of this group (overlaps next group's PE)
                for r in range(n0 // P, (n0 + nsz) // P):
                    ln_tile(y1[:, r, :], g2_b, b2_b, xn2T, r, scr2, stat2, tp4)
        close_pool(hT_cm)

        # ============ Phase 6: FFN up + gelu ============
        ff1_cm, ff1_pool = open_pool("ff1T", 1)
        ff1T = ff1_pool.tile([P, FFT, R], BF16, tag="ff1T")
        with tc.tile_pool(name="w_1", bufs=6) as wpool6, \
             tc.tile_pool(name="mm6", bufs=8, space="PSUM") as mm6:
            for fb in range(0, FFT, 2):
                groups = _ngroups(R)
                pss = {}
                for mi in range(2):
                    for ni, (n0, nsz) in enumerate(groups):
                        pss[(mi, ni)] = mm6.tile([P, nsz], F32, name="mm6",
                                                 tag="mm6")
                for k in range(FT):
                    wb = wchunk(wpool6, W1, k, fb * P, 2 * P, "w1",
                                eng="vector")
                    for mi in range(2):
                        for ni, (n0, nsz) in enumerate(groups):
                            nc.tensor.matmul(pss[(mi, ni)][:],
                                             wb[:, mi * P:(mi + 1) * P],
                                             xn2T[:, k, n0:n0 + nsz],
                                             start=(k == 0), stop=(k == FT - 1))
                for mi in range(2):
                    f = fb + mi
                    for ni, (n0, nsz) in enumerate(groups):
                        if not sim_safe_gelu:
                            nc.scalar.activation(ff1T[:, f, n0:n0 + nsz],
                                                 pss[(mi, ni)][:], ACT.Gelu,
                                                 bias=bf1_t[:, f:f + 1])
                        else:
                            _gelu_tanh(nc, tc, ff1T[:, f, n0:n0 + nsz],
                                       pss[(mi, ni)][:], bf1_t[:, f:f + 1],
                                       P, nsz)

        # ============ Phase 7: FFN down + residual -> out ============
        with tc.tile_pool(name="w_2", bufs=6) as wpool7, \
             tc.tile_pool(name="ev7", bufs=3) as ev7, \
             tc.tile_pool(name="ob7", bufs=3) as ob7, \
             tc.tile_pool(name="mm7", bufs=4, space="PSUM") as mm7, \
             tc.tile_pool(name="tp7", bufs=3, space="PSUM") as tp7:
            for fb in range(0, FT, 2):
                groups = _ngroups(R)
                pss = {}
                for mi in range(2):
                    for ni, (n0, nsz) in enumerate(groups):
                        pss[(mi, ni)] = mm7.tile([P, nsz], F32, name="mm7",
                                                 tag="mm7")
                for k in range(FFT):
                    wb = wchunk(wpool7, W2, k, fb * P, 2 * P, "w2",
                                eng="scalar")
                    for mi in range(2):
                        for ni, (n0, nsz) in enumerate(groups):
                            nc.tensor.matmul(pss[(mi, ni)][:],
                                             wb[:, mi * P:(mi + 1) * P],
                                             ff1T[:, k, n0:n0 + nsz],
                                             start=(k == 0), stop=(k == FFT - 1))
                for mi in range(2):
                    f = fb + mi
                    for ni, (n0, nsz) in enumerate(groups):
                        nj = nsz // P
                        pe = ev7.tile([P, nsz], F32, name="pe7", tag="pe7")
                        nc.vector.tensor_scalar(pe[:], pss[(mi, ni)][:],
                                                bf2_t[:, f:f + 1], None,
                                                op0=OP.add)
                        tp = tp7.tile([P, nsz], F32, name="tp7", tag="tp7")
                        for j in range(nj):
                            nc.tensor.transpose(tp[:, j * P:(j + 1) * P],
                                                pe[:, j * P:(j + 1) * P],
                                                ident_f32[:])
                        ob = ob7.tile([P, nj, P], F32, name="ob", tag="ob")
                        nc.vector.tensor_tensor(
                            ob[:], tp.rearrange("p (j c) -> p j c", c=P),
                            y1[:, n0 // P:n0 // P + nj, f * P:(f + 1) * P],
                            op=OP.add)
                        nc.sync.dma_start(
                            out[n0:n0 + nsz, f * P:(f + 1) * P]
                            .rearrange("(j p) c -> p j c", p=P), ob[:])
        close_pool(ff1_cm)
        close_pool(xn2_cm)
        close_pool(y1_cm)
        close_pool(const_cm)

    nc.compile()
    return nc


def _gelu_tanh(nc, tc, out_ap, ps, bias_col, p, nsz):
    """CoreSim-safe tanh gelu: 0.5*x*(1+tanh(0.79788456*(x+0.044715*x^3)))."""
    with tc.tile_pool(name="gelu_scr", bufs=2) as gs:
        x = gs.tile([p, nsz], F32, tag="g_x", name="g_x")
        nc.vector.tensor_scalar(x[:], ps[:], bias_col, None, op0=OP.add)
        x3 = gs.tile([p, nsz], F32, tag="g_x3", name="g_x3")
        nc.vector.tensor_tensor(x3[:], x[:], x[:], op=OP.mult)
        nc.vector.tensor_tensor(x3[:], x3[:], x[:], op=OP.mult)
        nc.vector.tensor_scalar(x3[:], x3[:], 0.044715, None, op0=OP.mult)
        nc.vector.tensor_tensor(x3[:], x3[:], x[:], op=OP.add)
        th = gs.tile([p, nsz], F32, tag="g_th", name="g_th")
        nc.scalar.activation(th[:], x3[:], ACT.Tanh, scale=0.7978845608028654)
        nc.vector.tensor_scalar(th[:], th[:], 1.0, 0.5, op0=OP.add, op1=OP.mult)
        nc.vector.tensor_tensor(out_ap, x[:], th[:], op=OP.mult)


# ---------------- host-side driver ----------------

_COMPILED = {}

_B, _S, _D, _H, _E, _FF = 4, 2048, 1024, 16, 64, 4096
_NCORES = 8
_R = (_B * _S) // _NCORES          # 1024 own rows per core
_CPB = _NCORES // _B               # cores per batch


def _get_nc():
    key = "full"
    if key not in _COMPILED:
        _COMPILED[key] = build_nc(R=_R, RB=_S, D=_D, H=_H, E=_E, FF=_FF,
                                  n_cores=_NCORES)
    return _COMPILED[key]


_WNAMES = ["Wq", "Wk", "Wv", "Wo", "W1", "W2", "bq", "bk", "bv", "bo",
           "bf1", "bf2", "g1", "b1", "g2", "b2"]


def kernel(**inputs):
    nc = _get_nc()
    x = np.ascontiguousarray(np.asarray(inputs["x"], dtype=np.float32))
    xf = x.reshape(_NCORES, _R, _D)
    xb = x.reshape(_B, _S, _D)
    shared = {n: np.ascontiguousarray(np.asarray(inputs[n], dtype=np.float32))
              for n in _WNAMES}
    in_maps = []
    for c in range(_NCORES):
        m = dict(shared)
        m["x_own"] = xf[c]
        m["x_batch"] = xb[c // _CPB]
        in_maps.append(m)
    res = run_bass_kernel_spmd(nc, in_maps, core_ids=list(range(_NCORES)))
    out = np.concatenate([res.results[c]["out"] for c in range(_NCORES)], axis=0)
    return out.reshape(_B, _S, _D).astype(np.float32)
